# revision 12
# baseline (speedup 1.0000x reference)
"""CenterLossLayer Trainium2 kernel (8-core SPMD, Bass/Tile).

Strategy: shard by LABEL RANGE (12500 classes per core). The host sorts
samples by label (pure index manipulation), packs them into 128-slot tiles
such that no class straddles a tile boundary, and hands each core its
samples in sorted order. All arithmetic (center gather, counts, group sums,
center update, loss) happens on device:

  per tile of 128 sorted samples:
    E[j,k]   = (first_slot_of_group(j) == k)   (DVE is_equal vs const iota)
    d0       = centers[label] - features       (rows via dma_gather)
    loss_j   = sum_d d0^2                      (ACT Square + accum)
    S        = E^T @ [d0 || 1]                 (PE f32: group sums + counts,
                                                landing at first-occurrence slots)
    value_k  = -ALPHA/(1+n_k) * S_k
  dma_scatter_add adds value rows into new_centers (pre-initialized with
  centers). First-occurrence slots target their class row (globally unique
  -> no RMW races); all other slots target discarded dump rows.

kernel(**inputs) takes FULL inputs, returns (result[B,1], new_centers[C,D]).
"""

import sys

sys.path.insert(0, "/opt/trn_rl_repo")

import numpy as np

from concourse import bass, bacc, mybir
import concourse.tile as tile
from concourse.bass_utils import run_bass_kernel_spmd

ALPHA = 0.5
NUM_CLASSES = 100000
FEAT_DIM = 128
BATCH = 131072
NCORES = 8

P = 128


class Cfg:
    def __init__(self, num_classes, batch, ncores, s_pad, dump_rows=1536):
        assert num_classes % ncores == 0
        self.C = num_classes
        self.B = batch
        self.ncores = ncores
        self.csh = num_classes // ncores  # classes per core
        assert s_pad % 2048 == 0
        self.s_pad = s_pad              # padded slots per core
        self.n_tiles = s_pad // P       # tiles per core
        self.dump = dump_rows           # discard rows appended to the table
        self.tbl = self.csh + dump_rows # per-core table rows
        self.n_chunks = s_pad // 2048   # dma chunks (16 tiles each)


# ----------------------------------------------------------------------------
# device program
# ----------------------------------------------------------------------------

def build_program(cfg: Cfg):
    nc = bacc.Bacc("TRN2", target_bir_lowering=False, debug=False,
                   num_devices=cfg.ncores)
    f32 = mybir.dt.float32
    i16 = mybir.dt.int16
    T = cfg.n_tiles
    TPC = 16  # tiles per dma chunk
    GPT = 8   # tiles per dma_gather call (1024 idxs; 2048 overflows the
    #           SWDGE descriptor ring on HW)
    idx_cols = cfg.s_pad // 16

    feat = nc.declare_dram_parameter("feat", [P, T * P], f32, isOutput=False)
    ctr = nc.declare_dram_parameter("ctr", [cfg.tbl, P], f32, isOutput=False)
    gidx = nc.declare_dram_parameter("gidx", [P, idx_cols], i16, isOutput=False)
    sidx = nc.declare_dram_parameter("sidx", [P, idx_cols], i16, isOutput=False)
    foT = nc.declare_dram_parameter("foT", [P, T], f32, isOutput=False)
    iota_in = nc.declare_dram_parameter("iota", [P, P], f32, isOutput=False)
    nctr = nc.declare_dram_parameter("nctr", [cfg.tbl, P], f32, isOutput=True)
    lout = nc.declare_dram_parameter("lout", [P, T], f32, isOutput=True)

    with tile.TileContext(nc) as tc:
        with (
            tc.tile_pool(name="const", bufs=1) as cpool,
            tc.tile_pool(name="io", bufs=3) as iopool,
            tc.tile_pool(name="work", bufs=3) as wpool,
            tc.tile_pool(name="sct", bufs=3) as spool,
            tc.tile_pool(name="ps_m", bufs=4, space="PSUM") as psm,
        ):
            # small constant loads first (sync HWDGE ring)
            iota = cpool.tile([P, P], f32)
            nc.sync.dma_start(out=iota[:], in_=iota_in[:])
            fo = cpool.tile([P, T], f32)
            nc.sync.dma_start(out=fo[:], in_=foT[:])
            gix = cpool.tile([P, idx_cols], i16)
            nc.sync.dma_start(out=gix[:], in_=gidx[:])
            six = cpool.tile([P, idx_cols], i16)
            nc.sync.dma_start(out=six[:], in_=sidx[:])
            losbuf = cpool.tile([P, T], f32)

            # new_centers := centers  (dense init on the scalar HWDGE ring so
            # it doesn't head-block the small loads / feature streams)
            nc.scalar.dma_start(out=nctr[:], in_=ctr[:])

            for kc in range(cfg.n_chunks):
                sct = spool.tile([P, TPC, P], f32, tag="sct")
                fk = iopool.tile([P, TPC, P], f32, tag="fk")
                nc.sync.dma_start(
                    out=fk[:], in_=feat[:, kc * TPC * P : (kc + 1) * TPC * P]
                )
                ck = iopool.tile([P, TPC, P], f32, tag="ck")
                for tt in range(0, TPC, GPT):
                    nc.gpsimd.dma_gather(
                        out_ap=ck[:, tt : tt + GPT, :],
                        in_ap=ctr[:],
                        idxs_ap=gix[:, (kc * TPC + tt) * 8 : (kc * TPC + tt + GPT) * 8],
                        num_idxs=GPT * P,
                        num_idxs_reg=GPT * P,
                        elem_size=P,
                    )
                for t in range(TPC):
                    gt = kc * TPC + t  # global tile id
                    # E[j,k] = (first_slot(j) == k)
                    E = wpool.tile([P, P], f32, tag="E")
                    nc.vector.tensor_tensor(
                        out=E[:],
                        in0=fo[:, gt : gt + 1].to_broadcast([P, P]),
                        in1=iota[:],
                        op=mybir.AluOpType.is_equal,
                    )
                    # d0e = [centers_row - feature || 1]
                    d0e = wpool.tile([P, P + 1], f32, tag="d0e")
                    nc.vector.memset(d0e[:, P : P + 1], 1.0)
                    nc.vector.tensor_tensor(
                        out=d0e[:, :P],
                        in0=ck[:, t, :],
                        in1=fk[:, t, :],
                        op=mybir.AluOpType.subtract,
                    )
                    # loss = sum(d0^2) along free dim (ACT square+accum)
                    sq = wpool.tile([P, P], f32, tag="sq")
                    nc.scalar.activation(
                        out=sq[:],
                        in_=d0e[:, :P],
                        func=mybir.ActivationFunctionType.Square,
                        accum_out=losbuf[:, gt : gt + 1],
                    )
                    # S = E^T @ [d0 || 1]: group sums + counts at first slots
                    S = psm.tile([P, P + 1], f32, space="PSUM", tag="S")
                    nc.tensor.matmul(
                        out=S[:], lhsT=E[:], rhs=d0e[:], start=True, stop=True
                    )
                    # r = 1/(1 + n)
                    n1 = wpool.tile([P, 1], f32, tag="n1")
                    nc.vector.tensor_scalar(
                        out=n1[:], in0=S[:, P : P + 1], scalar1=1.0, scalar2=None,
                        op0=mybir.AluOpType.add,
                    )
                    rv = wpool.tile([P, 1], f32, tag="rv")
                    nc.vector.reciprocal(out=rv[:], in_=n1[:])
                    # scatter value rows: -ALPHA * S * r
                    nc.vector.scalar_tensor_tensor(
                        out=sct[:, t, :],
                        in0=S[:, :P],
                        scalar=-ALPHA,
                        in1=rv[:].to_broadcast([P, P]),
                        op0=mybir.AluOpType.mult,
                        op1=mybir.AluOpType.mult,
                    )
                nc.gpsimd.dma_scatter_add(
                    out_ap=nctr[:],
                    in_ap=sct[:],
                    idxs_ap=six[:, kc * TPC * 8 : (kc + 1) * TPC * 8],
                    num_idxs=TPC * P,
                    num_idxs_reg=TPC * P,
                    elem_size=P,
                )
            nc.scalar.dma_start(out=lout[:], in_=losbuf[:])
    nc.finalize()
    return nc


# ----------------------------------------------------------------------------
# host sharding / packing
# ----------------------------------------------------------------------------

def host_pack(labels: np.ndarray, ncores: int, csh: int):
    """Sort by label, range-shard, pack into straddle-free 128-slot tiles.

    Returns (cores metadata list, s_pad). Pure index manipulation.
    """
    labels = np.asarray(labels).reshape(-1).astype(np.int64)
    order = np.argsort(labels, kind="stable")
    slab = labels[order]
    bounds = np.searchsorted(slab, np.arange(ncores + 1) * csh)
    packed = []
    used_max = 0
    for c in range(ncores):
        lo, hi = bounds[c], bounds[c + 1]
        samp = order[lo:hi]            # original sample idx, sorted by label
        lab = slab[lo:hi] - c * csh    # local labels, ascending
        n = lab.shape[0]
        starts = np.flatnonzero(np.r_[True, lab[1:] != lab[:-1]])
        lens = np.diff(np.r_[starts, n])
        assert lens.max(initial=0) <= P, "class run exceeds one tile"
        slot = np.empty(n, np.int64)
        cur = 0
        for s, L in zip(starts.tolist(), lens.tolist()):
            room = P - (cur % P)
            if L > room:
                cur += room
            slot[s : s + L] = np.arange(cur, cur + L)
            cur += L
        packed.append((samp, lab, starts, slot, cur))
        used_max = max(used_max, cur)

    s_pad = -(-used_max // 2048) * 2048
    cores = []
    for c in range(ncores):
        samp, lab, starts, slot, cur = packed[c]
        samp_at = np.full(s_pad, -1, np.int64)
        samp_at[slot] = samp
        real = samp_at >= 0

        gidx = np.zeros(s_pad, np.int16)
        gidx[slot] = lab.astype(np.int16)

        # first-slot-offset within tile, per slot; pads are singletons
        sl = np.arange(s_pad)
        fo = (sl % P).astype(np.int64)
        fo_real = np.empty(len(slot), np.int64)
        fo_real[:] = slot[starts].repeat(np.diff(np.r_[starts, len(slot)]))
        fo[slot] = fo_real % P

        first = np.zeros(s_pad, bool)
        first[slot[starts]] = True

        cores.append(
            dict(samp_at=samp_at, real=real, gidx=gidx,
                 fo=fo.astype(np.float32), first=first,
                 lab_first=lab[starts].astype(np.int16),
                 slot_first=slot[starts])
        )
    return cores, s_pad


def _wrap_idx(a: np.ndarray) -> np.ndarray:
    """[S] int16 -> [128, S/16] wrapped layout replicated to 8 groups."""
    w = a.reshape(-1, 16).T  # [16, S/16]
    return np.tile(w, (8, 1)).copy()


def make_in_maps(features, centers, cores, cfg: Cfg):
    features = np.asarray(features, dtype=np.float32)
    centers = np.asarray(centers, dtype=np.float32)
    T = cfg.n_tiles
    in_maps = []
    iota = np.tile(np.arange(P, dtype=np.float32), (P, 1))
    for c, m in enumerate(cores):
        fs = np.zeros((cfg.s_pad, P), np.float32)
        fs[m["real"]] = features[m["samp_at"][m["real"]]]
        feat_sw = np.ascontiguousarray(
            fs.reshape(T, P, P).transpose(1, 0, 2).reshape(P, T * P)
        )
        ctab = np.zeros((cfg.tbl, P), np.float32)
        ctab[: cfg.csh] = centers[c * cfg.csh : (c + 1) * cfg.csh]

        sl = np.arange(cfg.s_pad)
        sct = (cfg.csh + (sl % cfg.dump)).astype(np.int16)
        sct[m["slot_first"]] = m["lab_first"]

        in_maps.append(
            {
                "feat": feat_sw,
                "ctr": ctab,
                "gidx": _wrap_idx(m["gidx"]),
                "sidx": _wrap_idx(sct),
                "foT": np.ascontiguousarray(m["fo"].reshape(T, P).T),
                "iota": iota,
            }
        )
    return in_maps


def unshard(results, cores, cfg: Cfg):
    result = np.empty((cfg.B, 1), np.float32)
    new_centers = np.empty((cfg.C, P), np.float32)
    for c, (res, m) in enumerate(zip(results, cores)):
        new_centers[c * cfg.csh : (c + 1) * cfg.csh] = res["nctr"][: cfg.csh]
        loss_sorted = res["lout"].T.reshape(cfg.s_pad)  # slot i = [i%128, i//128]
        real = m["real"]
        result[m["samp_at"][real], 0] = loss_sorted[real]
    return result, new_centers


# ----------------------------------------------------------------------------
# entry point
# ----------------------------------------------------------------------------

_NC_CACHE = {}


def _get_nc(cfg: Cfg):
    key = (cfg.C, cfg.B, cfg.s_pad)
    if key not in _NC_CACHE:
        _NC_CACHE[key] = build_program(cfg)
    return _NC_CACHE[key]


def run(features, labels, centers, num_classes=NUM_CLASSES, dump_rows=1536,
        **spmd_kwargs):
    cores, s_pad = host_pack(labels, NCORES, num_classes // NCORES)
    cfg = Cfg(num_classes, len(np.asarray(labels).reshape(-1)), NCORES, s_pad,
              dump_rows=dump_rows)
    in_maps = make_in_maps(features, centers, cores, cfg)
    nc = _get_nc(cfg)
    br = run_bass_kernel_spmd(nc, in_maps, list(range(cfg.ncores)), **spmd_kwargs)
    result, new_centers = unshard(br.results, cores, cfg)
    return result, new_centers, br


def kernel(features, labels, centers):
    result, new_centers, _ = run(features, labels, centers)
    return result, new_centers


# revision 13
# speedup vs baseline: 1.1864x; 1.1864x over previous
"""CenterLossLayer Trainium2 kernel (8-core SPMD, Bass/Tile).

Strategy: shard by LABEL RANGE (12500 classes per core). The host sorts
samples by label (pure index manipulation), packs them into 128-slot tiles
such that no class straddles a tile boundary, and hands each core its
samples in sorted order. All arithmetic (center gather, counts, group sums,
center update, loss) happens on device:

  per tile of 128 sorted samples:
    E[j,k]   = (first_slot_of_group(j) == k)   (DVE is_equal vs const iota)
    d0       = centers[label] - features       (rows via dma_gather)
    loss_j   = sum_d d0^2                      (ACT Square + accum)
    S        = E^T @ [d0 || 1]                 (PE f32: group sums + counts,
                                                landing at first-occurrence slots)
    value_k  = -ALPHA/(1+n_k) * S_k
  dma_scatter_add adds value rows into new_centers (pre-initialized with
  centers). First-occurrence slots target their class row (globally unique
  -> no RMW races); all other slots target discarded dump rows.

kernel(**inputs) takes FULL inputs, returns (result[B,1], new_centers[C,D]).
"""

import sys

sys.path.insert(0, "/opt/trn_rl_repo")

import numpy as np

from concourse import bass, bacc, mybir
import concourse.tile as tile
from concourse.bass_utils import run_bass_kernel_spmd

ALPHA = 0.5
NUM_CLASSES = 100000
FEAT_DIM = 128
BATCH = 131072
NCORES = 8

P = 128


class Cfg:
    def __init__(self, num_classes, batch, ncores, s_pad, dump_rows=1536):
        assert num_classes % ncores == 0
        self.C = num_classes
        self.B = batch
        self.ncores = ncores
        self.csh = num_classes // ncores  # classes per core
        assert s_pad % 2048 == 0
        self.s_pad = s_pad              # padded slots per core
        self.n_tiles = s_pad // P       # tiles per core
        self.dump = dump_rows           # discard rows appended to the table
        self.tbl = self.csh + dump_rows # per-core table rows
        self.n_chunks = s_pad // 2048   # dma chunks (16 tiles each)


# ----------------------------------------------------------------------------
# device program
# ----------------------------------------------------------------------------

def build_program(cfg: Cfg):
    nc = bacc.Bacc("TRN2", target_bir_lowering=False, debug=False,
                   num_devices=cfg.ncores)
    f32 = mybir.dt.float32
    i16 = mybir.dt.int16
    T = cfg.n_tiles
    TPC = 16  # tiles per dma chunk
    GPT = 8   # tiles per dma_gather call (1024 idxs; 2048 overflows the
    #           SWDGE descriptor ring on HW)
    idx_cols = cfg.s_pad // 16

    feat = nc.declare_dram_parameter("feat", [P, T * P], f32, isOutput=False)
    ctr = nc.declare_dram_parameter("ctr", [cfg.tbl, P], f32, isOutput=False)
    gidx = nc.declare_dram_parameter("gidx", [P, idx_cols], i16, isOutput=False)
    sidx = nc.declare_dram_parameter("sidx", [P, idx_cols], i16, isOutput=False)
    foT = nc.declare_dram_parameter("foT", [P, T], f32, isOutput=False)
    iota_in = nc.declare_dram_parameter("iota", [P, P], f32, isOutput=False)
    nctr = nc.declare_dram_parameter("nctr", [cfg.tbl, P], f32, isOutput=True)
    lout = nc.declare_dram_parameter("lout", [P, T], f32, isOutput=True)

    with tile.TileContext(nc) as tc:
        with (
            tc.tile_pool(name="const", bufs=1) as cpool,
            tc.tile_pool(name="io", bufs=3) as iopool,
            tc.tile_pool(name="work", bufs=3) as wpool,
            tc.tile_pool(name="sct", bufs=3) as spool,
            tc.tile_pool(name="ps_m", bufs=4, space="PSUM") as psm,
        ):
            # small constant loads first (sync HWDGE ring)
            iota = cpool.tile([P, P], f32)
            nc.sync.dma_start(out=iota[:], in_=iota_in[:])
            fo = cpool.tile([P, T], f32)
            nc.sync.dma_start(out=fo[:], in_=foT[:])
            gix = cpool.tile([P, idx_cols], i16)
            nc.sync.dma_start(out=gix[:], in_=gidx[:])
            six = cpool.tile([P, idx_cols], i16)
            nc.sync.dma_start(out=six[:], in_=sidx[:])
            losbuf = cpool.tile([P, T], f32)

            # new_centers := centers. Emitted on the same HWDGE ring AFTER the
            # small index loads (FIFO per ring), so the gathers can start
            # within a few us; dump rows rely on the zero-initialized output
            # buffer. The first scatter-add lands long after this finishes.
            nc.sync.dma_start(out=nctr[: cfg.csh], in_=ctr[: cfg.csh])

            for kc in range(cfg.n_chunks):
                sct = spool.tile([P, TPC, P], f32, tag="sct")
                fk = iopool.tile([P, TPC, P], f32, tag="fk")
                nc.sync.dma_start(
                    out=fk[:], in_=feat[:, kc * TPC * P : (kc + 1) * TPC * P]
                )
                ck = iopool.tile([P, TPC, P], f32, tag="ck")
                for tt in range(0, TPC, GPT):
                    nc.gpsimd.dma_gather(
                        out_ap=ck[:, tt : tt + GPT, :],
                        in_ap=ctr[:],
                        idxs_ap=gix[:, (kc * TPC + tt) * 8 : (kc * TPC + tt + GPT) * 8],
                        num_idxs=GPT * P,
                        num_idxs_reg=GPT * P,
                        elem_size=P,
                    )
                for t in range(TPC):
                    gt = kc * TPC + t  # global tile id
                    # E[j,k] = (first_slot(j) == k)
                    E = wpool.tile([P, P], f32, tag="E")
                    nc.vector.tensor_tensor(
                        out=E[:],
                        in0=fo[:, gt : gt + 1].to_broadcast([P, P]),
                        in1=iota[:],
                        op=mybir.AluOpType.is_equal,
                    )
                    # d0e = [centers_row - feature || 1]
                    d0e = wpool.tile([P, P + 1], f32, tag="d0e")
                    nc.vector.memset(d0e[:, P : P + 1], 1.0)
                    nc.vector.tensor_tensor(
                        out=d0e[:, :P],
                        in0=ck[:, t, :],
                        in1=fk[:, t, :],
                        op=mybir.AluOpType.subtract,
                    )
                    # loss = sum(d0^2) along free dim (ACT square+accum)
                    sq = wpool.tile([P, P], f32, tag="sq")
                    nc.scalar.activation(
                        out=sq[:],
                        in_=d0e[:, :P],
                        func=mybir.ActivationFunctionType.Square,
                        accum_out=losbuf[:, gt : gt + 1],
                    )
                    # S = E^T @ [d0 || 1]: group sums + counts at first slots
                    S = psm.tile([P, P + 1], f32, space="PSUM", tag="S")
                    nc.tensor.matmul(
                        out=S[:], lhsT=E[:], rhs=d0e[:], start=True, stop=True
                    )
                    # r = 1/(1 + n)
                    n1 = wpool.tile([P, 1], f32, tag="n1")
                    nc.vector.tensor_scalar(
                        out=n1[:], in0=S[:, P : P + 1], scalar1=1.0, scalar2=None,
                        op0=mybir.AluOpType.add,
                    )
                    rv = wpool.tile([P, 1], f32, tag="rv")
                    nc.vector.reciprocal(out=rv[:], in_=n1[:])
                    # scatter value rows: -ALPHA * S * r
                    nc.vector.scalar_tensor_tensor(
                        out=sct[:, t, :],
                        in0=S[:, :P],
                        scalar=-ALPHA,
                        in1=rv[:].to_broadcast([P, P]),
                        op0=mybir.AluOpType.mult,
                        op1=mybir.AluOpType.mult,
                    )
                nc.gpsimd.dma_scatter_add(
                    out_ap=nctr[:],
                    in_ap=sct[:],
                    idxs_ap=six[:, kc * TPC * 8 : (kc + 1) * TPC * 8],
                    num_idxs=TPC * P,
                    num_idxs_reg=TPC * P,
                    elem_size=P,
                )
            nc.scalar.dma_start(out=lout[:], in_=losbuf[:])
    nc.finalize()
    return nc


# ----------------------------------------------------------------------------
# host sharding / packing
# ----------------------------------------------------------------------------

def host_pack(labels: np.ndarray, ncores: int, csh: int):
    """Sort by label, range-shard, pack into straddle-free 128-slot tiles.

    Returns (cores metadata list, s_pad). Pure index manipulation.
    """
    labels = np.asarray(labels).reshape(-1).astype(np.int64)
    order = np.argsort(labels, kind="stable")
    slab = labels[order]
    bounds = np.searchsorted(slab, np.arange(ncores + 1) * csh)
    packed = []
    used_max = 0
    for c in range(ncores):
        lo, hi = bounds[c], bounds[c + 1]
        samp = order[lo:hi]            # original sample idx, sorted by label
        lab = slab[lo:hi] - c * csh    # local labels, ascending
        n = lab.shape[0]
        starts = np.flatnonzero(np.r_[True, lab[1:] != lab[:-1]])
        lens = np.diff(np.r_[starts, n])
        assert lens.max(initial=0) <= P, "class run exceeds one tile"
        slot = np.empty(n, np.int64)
        cur = 0
        for s, L in zip(starts.tolist(), lens.tolist()):
            room = P - (cur % P)
            if L > room:
                cur += room
            slot[s : s + L] = np.arange(cur, cur + L)
            cur += L
        packed.append((samp, lab, starts, slot, cur))
        used_max = max(used_max, cur)

    s_pad = -(-used_max // 2048) * 2048
    cores = []
    for c in range(ncores):
        samp, lab, starts, slot, cur = packed[c]
        samp_at = np.full(s_pad, -1, np.int64)
        samp_at[slot] = samp
        real = samp_at >= 0

        gidx = np.zeros(s_pad, np.int16)
        gidx[slot] = lab.astype(np.int16)

        # first-slot-offset within tile, per slot; pads are singletons
        sl = np.arange(s_pad)
        fo = (sl % P).astype(np.int64)
        fo_real = np.empty(len(slot), np.int64)
        fo_real[:] = slot[starts].repeat(np.diff(np.r_[starts, len(slot)]))
        fo[slot] = fo_real % P

        first = np.zeros(s_pad, bool)
        first[slot[starts]] = True

        cores.append(
            dict(samp_at=samp_at, real=real, gidx=gidx,
                 fo=fo.astype(np.float32), first=first,
                 lab_first=lab[starts].astype(np.int16),
                 slot_first=slot[starts])
        )
    return cores, s_pad


def _wrap_idx(a: np.ndarray) -> np.ndarray:
    """[S] int16 -> [128, S/16] wrapped layout replicated to 8 groups."""
    w = a.reshape(-1, 16).T  # [16, S/16]
    return np.tile(w, (8, 1)).copy()


def make_in_maps(features, centers, cores, cfg: Cfg):
    features = np.asarray(features, dtype=np.float32)
    centers = np.asarray(centers, dtype=np.float32)
    T = cfg.n_tiles
    in_maps = []
    iota = np.tile(np.arange(P, dtype=np.float32), (P, 1))
    for c, m in enumerate(cores):
        fs = np.zeros((cfg.s_pad, P), np.float32)
        fs[m["real"]] = features[m["samp_at"][m["real"]]]
        feat_sw = np.ascontiguousarray(
            fs.reshape(T, P, P).transpose(1, 0, 2).reshape(P, T * P)
        )
        ctab = np.zeros((cfg.tbl, P), np.float32)
        ctab[: cfg.csh] = centers[c * cfg.csh : (c + 1) * cfg.csh]

        sl = np.arange(cfg.s_pad)
        sct = (cfg.csh + (sl % cfg.dump)).astype(np.int16)
        sct[m["slot_first"]] = m["lab_first"]

        in_maps.append(
            {
                "feat": feat_sw,
                "ctr": ctab,
                "gidx": _wrap_idx(m["gidx"]),
                "sidx": _wrap_idx(sct),
                "foT": np.ascontiguousarray(m["fo"].reshape(T, P).T),
                "iota": iota,
            }
        )
    return in_maps


def unshard(results, cores, cfg: Cfg):
    result = np.empty((cfg.B, 1), np.float32)
    new_centers = np.empty((cfg.C, P), np.float32)
    for c, (res, m) in enumerate(zip(results, cores)):
        new_centers[c * cfg.csh : (c + 1) * cfg.csh] = res["nctr"][: cfg.csh]
        loss_sorted = res["lout"].T.reshape(cfg.s_pad)  # slot i = [i%128, i//128]
        real = m["real"]
        result[m["samp_at"][real], 0] = loss_sorted[real]
    return result, new_centers


# ----------------------------------------------------------------------------
# entry point
# ----------------------------------------------------------------------------

_NC_CACHE = {}


def _get_nc(cfg: Cfg):
    key = (cfg.C, cfg.B, cfg.s_pad)
    if key not in _NC_CACHE:
        _NC_CACHE[key] = build_program(cfg)
    return _NC_CACHE[key]


def run(features, labels, centers, num_classes=NUM_CLASSES, dump_rows=1536,
        **spmd_kwargs):
    cores, s_pad = host_pack(labels, NCORES, num_classes // NCORES)
    cfg = Cfg(num_classes, len(np.asarray(labels).reshape(-1)), NCORES, s_pad,
              dump_rows=dump_rows)
    in_maps = make_in_maps(features, centers, cores, cfg)
    nc = _get_nc(cfg)
    br = run_bass_kernel_spmd(nc, in_maps, list(range(cfg.ncores)), **spmd_kwargs)
    result, new_centers = unshard(br.results, cores, cfg)
    return result, new_centers, br


def kernel(features, labels, centers):
    result, new_centers, _ = run(features, labels, centers)
    return result, new_centers


# revision 16
# speedup vs baseline: 1.2821x; 1.0807x over previous
"""CenterLossLayer Trainium2 kernel (8-core SPMD, Bass/Tile).

Strategy: shard by LABEL RANGE (12500 classes per core). The host sorts
samples by label (pure index manipulation), packs them into 128-slot tiles
such that no class straddles a tile boundary, and hands each core its
samples in sorted order. All arithmetic (center gather, counts, group sums,
center update, loss) happens on device:

  per tile of 128 sorted samples:
    E[j,k]   = (first_slot_of_group(j) == k)   (DVE is_equal vs const iota)
    d0       = centers[label] - features       (rows via dma_gather)
    loss_j   = sum_d d0^2                      (ACT Square + accum)
    S        = E^T @ [d0 || 1]                 (PE f32: group sums + counts,
                                                landing at first-occurrence slots)
    value_k  = -ALPHA/(1+n_k) * S_k
  dma_scatter_add adds value rows into new_centers (pre-initialized with
  centers). First-occurrence slots target their class row (globally unique
  -> no RMW races); all other slots target discarded dump rows.

kernel(**inputs) takes FULL inputs, returns (result[B,1], new_centers[C,D]).
"""

import sys

sys.path.insert(0, "/opt/trn_rl_repo")

import numpy as np

from concourse import bass, bacc, mybir
import concourse.tile as tile
from concourse.bass_utils import run_bass_kernel_spmd

ALPHA = 0.5
NUM_CLASSES = 100000
FEAT_DIM = 128
BATCH = 131072
NCORES = 8

P = 128


class Cfg:
    def __init__(self, num_classes, batch, ncores, s_pad, dump_rows=1536):
        assert num_classes % ncores == 0
        self.C = num_classes
        self.B = batch
        self.ncores = ncores
        self.csh = num_classes // ncores  # classes per core
        assert s_pad % 2048 == 0
        self.s_pad = s_pad              # padded slots per core
        self.n_tiles = s_pad // P       # tiles per core
        self.dump = dump_rows           # discard rows appended to the table
        self.tbl = self.csh + dump_rows # per-core table rows
        self.n_chunks = s_pad // 2048   # dma chunks (16 tiles each)


# ----------------------------------------------------------------------------
# device program
# ----------------------------------------------------------------------------

def build_program(cfg: Cfg):
    nc = bacc.Bacc("TRN2", target_bir_lowering=False, debug=False,
                   num_devices=cfg.ncores, num_swdge_queues=4)
    f32 = mybir.dt.float32
    i16 = mybir.dt.int16
    T = cfg.n_tiles
    TPC = 16  # tiles per dma chunk
    GPT = 8   # tiles per dma_gather call (1024 idxs; 2048 overflows the
    #           SWDGE descriptor ring on HW)
    idx_cols = cfg.s_pad // 16

    feat = nc.declare_dram_parameter("feat", [P, T * P], f32, isOutput=False)
    ctr = nc.declare_dram_parameter("ctr", [cfg.tbl, P], f32, isOutput=False)
    gidx = nc.declare_dram_parameter("gidx", [P, idx_cols], i16, isOutput=False)
    sidx = nc.declare_dram_parameter("sidx", [P, idx_cols], i16, isOutput=False)
    foT = nc.declare_dram_parameter("foT", [P, T], f32, isOutput=False)
    iota_in = nc.declare_dram_parameter("iota", [P, P], f32, isOutput=False)
    nctr = nc.declare_dram_parameter("nctr", [cfg.tbl, P], f32, isOutput=True)
    lout = nc.declare_dram_parameter("lout", [P, T], f32, isOutput=True)

    with tile.TileContext(nc) as tc:
        with (
            tc.tile_pool(name="const", bufs=1) as cpool,
            tc.tile_pool(name="io", bufs=3) as iopool,
            tc.tile_pool(name="work", bufs=3) as wpool,
            tc.tile_pool(name="sct", bufs=3) as spool,
            tc.tile_pool(name="ps_m", bufs=4, space="PSUM") as psm,
        ):
            # small constant loads first (sync HWDGE ring)
            iota = cpool.tile([P, P], f32)
            nc.sync.dma_start(out=iota[:], in_=iota_in[:])
            fo = cpool.tile([P, T], f32)
            nc.sync.dma_start(out=fo[:], in_=foT[:])
            gix = cpool.tile([P, idx_cols], i16)
            nc.sync.dma_start(out=gix[:], in_=gidx[:])
            six = cpool.tile([P, idx_cols], i16)
            nc.sync.dma_start(out=six[:], in_=sidx[:])
            losbuf = cpool.tile([P, T], f32)

            # new_centers := centers. Emitted on the same HWDGE ring AFTER the
            # small index loads (FIFO per ring), so the gathers can start
            # within a few us; dump rows rely on the zero-initialized output
            # buffer. The first scatter-add lands long after this finishes.
            nc.sync.dma_start(out=nctr[: cfg.csh], in_=ctr[: cfg.csh])

            for kc in range(cfg.n_chunks):
                sct = spool.tile([P, TPC, P], f32, tag="sct")
                fk = iopool.tile([P, TPC, P], f32, tag="fk")
                nc.sync.dma_start(
                    out=fk[:], in_=feat[:, kc * TPC * P : (kc + 1) * TPC * P]
                )
                ck = iopool.tile([P, TPC, P], f32, tag="ck")
                for tt in range(0, TPC, GPT):
                    # round-robin SWDGE queues: queue q runs on Q7 core pair q,
                    # letting descriptor generation pipeline across pairs
                    nc.gpsimd.dma_gather(
                        out_ap=ck[:, tt : tt + GPT, :],
                        in_ap=ctr[:],
                        idxs_ap=gix[:, (kc * TPC + tt) * 8 : (kc * TPC + tt + GPT) * 8],
                        num_idxs=GPT * P,
                        num_idxs_reg=GPT * P,
                        elem_size=P,
                        queue_num=(2 * kc + tt // GPT) % 3,
                    )
                for t in range(TPC):
                    gt = kc * TPC + t  # global tile id
                    # E[j,k] = (first_slot(j) == k)
                    E = wpool.tile([P, P], f32, tag="E")
                    nc.vector.tensor_tensor(
                        out=E[:],
                        in0=fo[:, gt : gt + 1].to_broadcast([P, P]),
                        in1=iota[:],
                        op=mybir.AluOpType.is_equal,
                    )
                    # d0e = [centers_row - feature || 1]
                    d0e = wpool.tile([P, P + 1], f32, tag="d0e")
                    nc.vector.memset(d0e[:, P : P + 1], 1.0)
                    nc.vector.tensor_tensor(
                        out=d0e[:, :P],
                        in0=ck[:, t, :],
                        in1=fk[:, t, :],
                        op=mybir.AluOpType.subtract,
                    )
                    # loss = sum(d0^2) along free dim (ACT square+accum)
                    sq = wpool.tile([P, P], f32, tag="sq")
                    nc.scalar.activation(
                        out=sq[:],
                        in_=d0e[:, :P],
                        func=mybir.ActivationFunctionType.Square,
                        accum_out=losbuf[:, gt : gt + 1],
                    )
                    # S = E^T @ [d0 || 1]: group sums + counts at first slots
                    S = psm.tile([P, P + 1], f32, space="PSUM", tag="S")
                    nc.tensor.matmul(
                        out=S[:], lhsT=E[:], rhs=d0e[:], start=True, stop=True
                    )
                    # r = 1/(1 + n)
                    n1 = wpool.tile([P, 1], f32, tag="n1")
                    nc.vector.tensor_scalar(
                        out=n1[:], in0=S[:, P : P + 1], scalar1=1.0, scalar2=None,
                        op0=mybir.AluOpType.add,
                    )
                    rv = wpool.tile([P, 1], f32, tag="rv")
                    nc.vector.reciprocal(out=rv[:], in_=n1[:])
                    # scatter value rows: -ALPHA * S * r
                    nc.vector.scalar_tensor_tensor(
                        out=sct[:, t, :],
                        in0=S[:, :P],
                        scalar=-ALPHA,
                        in1=rv[:].to_broadcast([P, P]),
                        op0=mybir.AluOpType.mult,
                        op1=mybir.AluOpType.mult,
                    )
                nc.gpsimd.dma_scatter_add(
                    out_ap=nctr[:],
                    in_ap=sct[:],
                    idxs_ap=six[:, kc * TPC * 8 : (kc + 1) * TPC * 8],
                    num_idxs=TPC * P,
                    num_idxs_reg=TPC * P,
                    elem_size=P,
                    queue_num=3,
                )
            nc.scalar.dma_start(out=lout[:], in_=losbuf[:])
    nc.finalize()
    return nc


# ----------------------------------------------------------------------------
# host sharding / packing
# ----------------------------------------------------------------------------

def host_pack(labels: np.ndarray, ncores: int, csh: int):
    """Sort by label, range-shard, pack into straddle-free 128-slot tiles.

    Returns (cores metadata list, s_pad). Pure index manipulation.
    """
    labels = np.asarray(labels).reshape(-1).astype(np.int64)
    order = np.argsort(labels, kind="stable")
    slab = labels[order]
    bounds = np.searchsorted(slab, np.arange(ncores + 1) * csh)
    packed = []
    used_max = 0
    for c in range(ncores):
        lo, hi = bounds[c], bounds[c + 1]
        samp = order[lo:hi]            # original sample idx, sorted by label
        lab = slab[lo:hi] - c * csh    # local labels, ascending
        n = lab.shape[0]
        starts = np.flatnonzero(np.r_[True, lab[1:] != lab[:-1]])
        lens = np.diff(np.r_[starts, n])
        assert lens.max(initial=0) <= P, "class run exceeds one tile"
        slot = np.empty(n, np.int64)
        cur = 0
        for s, L in zip(starts.tolist(), lens.tolist()):
            room = P - (cur % P)
            if L > room:
                cur += room
            slot[s : s + L] = np.arange(cur, cur + L)
            cur += L
        packed.append((samp, lab, starts, slot, cur))
        used_max = max(used_max, cur)

    s_pad = -(-used_max // 2048) * 2048
    cores = []
    for c in range(ncores):
        samp, lab, starts, slot, cur = packed[c]
        samp_at = np.full(s_pad, -1, np.int64)
        samp_at[slot] = samp
        real = samp_at >= 0

        gidx = np.zeros(s_pad, np.int16)
        gidx[slot] = lab.astype(np.int16)

        # first-slot-offset within tile, per slot; pads are singletons
        sl = np.arange(s_pad)
        fo = (sl % P).astype(np.int64)
        fo_real = np.empty(len(slot), np.int64)
        fo_real[:] = slot[starts].repeat(np.diff(np.r_[starts, len(slot)]))
        fo[slot] = fo_real % P

        first = np.zeros(s_pad, bool)
        first[slot[starts]] = True

        cores.append(
            dict(samp_at=samp_at, real=real, gidx=gidx,
                 fo=fo.astype(np.float32), first=first,
                 lab_first=lab[starts].astype(np.int16),
                 slot_first=slot[starts])
        )
    return cores, s_pad


def _wrap_idx(a: np.ndarray) -> np.ndarray:
    """[S] int16 -> [128, S/16] wrapped layout replicated to 8 groups."""
    w = a.reshape(-1, 16).T  # [16, S/16]
    return np.tile(w, (8, 1)).copy()


def make_in_maps(features, centers, cores, cfg: Cfg):
    features = np.asarray(features, dtype=np.float32)
    centers = np.asarray(centers, dtype=np.float32)
    T = cfg.n_tiles
    in_maps = []
    iota = np.tile(np.arange(P, dtype=np.float32), (P, 1))
    for c, m in enumerate(cores):
        fs = np.zeros((cfg.s_pad, P), np.float32)
        fs[m["real"]] = features[m["samp_at"][m["real"]]]
        feat_sw = np.ascontiguousarray(
            fs.reshape(T, P, P).transpose(1, 0, 2).reshape(P, T * P)
        )
        ctab = np.zeros((cfg.tbl, P), np.float32)
        ctab[: cfg.csh] = centers[c * cfg.csh : (c + 1) * cfg.csh]

        sl = np.arange(cfg.s_pad)
        sct = (cfg.csh + (sl % cfg.dump)).astype(np.int16)
        sct[m["slot_first"]] = m["lab_first"]

        in_maps.append(
            {
                "feat": feat_sw,
                "ctr": ctab,
                "gidx": _wrap_idx(m["gidx"]),
                "sidx": _wrap_idx(sct),
                "foT": np.ascontiguousarray(m["fo"].reshape(T, P).T),
                "iota": iota,
            }
        )
    return in_maps


def unshard(results, cores, cfg: Cfg):
    result = np.empty((cfg.B, 1), np.float32)
    new_centers = np.empty((cfg.C, P), np.float32)
    for c, (res, m) in enumerate(zip(results, cores)):
        new_centers[c * cfg.csh : (c + 1) * cfg.csh] = res["nctr"][: cfg.csh]
        loss_sorted = res["lout"].T.reshape(cfg.s_pad)  # slot i = [i%128, i//128]
        real = m["real"]
        result[m["samp_at"][real], 0] = loss_sorted[real]
    return result, new_centers


# ----------------------------------------------------------------------------
# entry point
# ----------------------------------------------------------------------------

_NC_CACHE = {}


def _get_nc(cfg: Cfg):
    key = (cfg.C, cfg.B, cfg.s_pad)
    if key not in _NC_CACHE:
        _NC_CACHE[key] = build_program(cfg)
    return _NC_CACHE[key]


def run(features, labels, centers, num_classes=NUM_CLASSES, dump_rows=1536,
        **spmd_kwargs):
    cores, s_pad = host_pack(labels, NCORES, num_classes // NCORES)
    cfg = Cfg(num_classes, len(np.asarray(labels).reshape(-1)), NCORES, s_pad,
              dump_rows=dump_rows)
    in_maps = make_in_maps(features, centers, cores, cfg)
    nc = _get_nc(cfg)
    br = run_bass_kernel_spmd(nc, in_maps, list(range(cfg.ncores)), **spmd_kwargs)
    result, new_centers = unshard(br.results, cores, cfg)
    return result, new_centers, br


def kernel(features, labels, centers):
    result, new_centers, _ = run(features, labels, centers)
    return result, new_centers


# revision 17
# speedup vs baseline: 1.4417x; 1.1244x over previous
"""CenterLossLayer Trainium2 kernel (8-core SPMD, Bass/Tile).

Strategy: shard by LABEL RANGE (12500 classes per core). The host sorts
samples by label (pure index manipulation), packs them into 128-slot tiles
such that no class straddles a tile boundary, and hands each core its
samples in sorted order. All arithmetic (center gather, counts, group sums,
center update, loss) happens on device:

  per tile of 128 sorted samples:
    E[j,k]   = (first_slot_of_group(j) == k)   (DVE is_equal vs const iota)
    d0       = centers[label] - features       (rows via dma_gather)
    loss_j   = sum_d d0^2                      (ACT Square + accum)
    S        = E^T @ [d0 || 1]                 (PE f32: group sums + counts,
                                                landing at first-occurrence slots)
    value_k  = -ALPHA/(1+n_k) * S_k
  dma_scatter_add adds value rows into new_centers (pre-initialized with
  centers). First-occurrence slots target their class row (globally unique
  -> no RMW races); all other slots target discarded dump rows.

kernel(**inputs) takes FULL inputs, returns (result[B,1], new_centers[C,D]).
"""

import sys

sys.path.insert(0, "/opt/trn_rl_repo")

import numpy as np

from concourse import bass, bacc, mybir
import concourse.tile as tile
from concourse.bass_utils import run_bass_kernel_spmd

ALPHA = 0.5
NUM_CLASSES = 100000
FEAT_DIM = 128
BATCH = 131072
NCORES = 8

P = 128


class Cfg:
    def __init__(self, num_classes, batch, ncores, s_pad, dump_rows=1536):
        assert num_classes % ncores == 0
        self.C = num_classes
        self.B = batch
        self.ncores = ncores
        self.csh = num_classes // ncores  # classes per core
        assert s_pad % 2048 == 0
        self.s_pad = s_pad              # padded slots per core
        self.n_tiles = s_pad // P       # tiles per core
        self.dump = dump_rows           # discard rows appended to the table
        self.tbl = self.csh + dump_rows # per-core table rows
        self.n_chunks = s_pad // 2048   # dma chunks (16 tiles each)


# ----------------------------------------------------------------------------
# device program
# ----------------------------------------------------------------------------

def build_program(cfg: Cfg):
    nc = bacc.Bacc("TRN2", target_bir_lowering=False, debug=False,
                   num_devices=cfg.ncores, num_swdge_queues=4)
    f32 = mybir.dt.float32
    i16 = mybir.dt.int16
    T = cfg.n_tiles
    TPC = 16  # tiles per dma chunk
    GPT = 8   # tiles per dma_gather call (1024 idxs; 2048 overflows the
    #           SWDGE descriptor ring on HW)
    idx_cols = cfg.s_pad // 16

    feat = nc.declare_dram_parameter("feat", [P, T * P], f32, isOutput=False)
    ctr = nc.declare_dram_parameter("ctr", [cfg.tbl, P], f32, isOutput=False)
    gidx = nc.declare_dram_parameter("gidx", [P, idx_cols], i16, isOutput=False)
    sidx = nc.declare_dram_parameter("sidx", [P, idx_cols], i16, isOutput=False)
    foT = nc.declare_dram_parameter("foT", [P, T], f32, isOutput=False)
    iota_in = nc.declare_dram_parameter("iota", [P, P], f32, isOutput=False)
    nctr = nc.declare_dram_parameter("nctr", [cfg.tbl, P], f32, isOutput=True)
    lout = nc.declare_dram_parameter("lout", [P, T], f32, isOutput=True)

    with tile.TileContext(nc) as tc:
        with (
            tc.tile_pool(name="const", bufs=1) as cpool,
            tc.tile_pool(name="io", bufs=5) as iopool,
            tc.tile_pool(name="work", bufs=3) as wpool,
            tc.tile_pool(name="sct", bufs=3) as spool,
            tc.tile_pool(name="ps_m", bufs=4, space="PSUM") as psm,
        ):
            # small constant loads first (sync HWDGE ring)
            iota = cpool.tile([P, P], f32)
            nc.sync.dma_start(out=iota[:], in_=iota_in[:])
            fo = cpool.tile([P, T], f32)
            nc.sync.dma_start(out=fo[:], in_=foT[:])
            gix = cpool.tile([P, idx_cols], i16)
            nc.sync.dma_start(out=gix[:], in_=gidx[:])
            six = cpool.tile([P, idx_cols], i16)
            nc.sync.dma_start(out=six[:], in_=sidx[:])
            losbuf = cpool.tile([P, T], f32)

            # new_centers := centers. Emitted on the same HWDGE ring AFTER the
            # small index loads (FIFO per ring), so the gathers can start
            # within a few us; dump rows rely on the zero-initialized output
            # buffer. The first scatter-add lands long after this finishes.
            nc.sync.dma_start(out=nctr[: cfg.csh], in_=ctr[: cfg.csh])

            for kc in range(cfg.n_chunks):
                sct = spool.tile([P, TPC, P], f32, tag="sct")
                fk = iopool.tile([P, TPC, P], f32, tag="fk")
                nc.sync.dma_start(
                    out=fk[:], in_=feat[:, kc * TPC * P : (kc + 1) * TPC * P]
                )
                ck = iopool.tile([P, TPC, P], f32, tag="ck")
                for tt in range(0, TPC, GPT):
                    # round-robin SWDGE queues: queue q runs on Q7 core pair q,
                    # letting descriptor generation pipeline across pairs
                    nc.gpsimd.dma_gather(
                        out_ap=ck[:, tt : tt + GPT, :],
                        in_ap=ctr[:],
                        idxs_ap=gix[:, (kc * TPC + tt) * 8 : (kc * TPC + tt + GPT) * 8],
                        num_idxs=GPT * P,
                        num_idxs_reg=GPT * P,
                        elem_size=P,
                        queue_num=(2 * kc + tt // GPT) % 3,
                    )
                for t in range(TPC):
                    gt = kc * TPC + t  # global tile id
                    # E[j,k] = (first_slot(j) == k)
                    E = wpool.tile([P, P], f32, tag="E")
                    nc.vector.tensor_tensor(
                        out=E[:],
                        in0=fo[:, gt : gt + 1].to_broadcast([P, P]),
                        in1=iota[:],
                        op=mybir.AluOpType.is_equal,
                    )
                    # d0e = [centers_row - feature || 1]
                    d0e = wpool.tile([P, P + 1], f32, tag="d0e")
                    nc.vector.memset(d0e[:, P : P + 1], 1.0)
                    nc.vector.tensor_tensor(
                        out=d0e[:, :P],
                        in0=ck[:, t, :],
                        in1=fk[:, t, :],
                        op=mybir.AluOpType.subtract,
                    )
                    # loss = sum(d0^2) along free dim (ACT square+accum)
                    sq = wpool.tile([P, P], f32, tag="sq")
                    nc.scalar.activation(
                        out=sq[:],
                        in_=d0e[:, :P],
                        func=mybir.ActivationFunctionType.Square,
                        accum_out=losbuf[:, gt : gt + 1],
                    )
                    # S = E^T @ [d0 || 1]: group sums + counts at first slots
                    S = psm.tile([P, P + 1], f32, space="PSUM", tag="S")
                    nc.tensor.matmul(
                        out=S[:], lhsT=E[:], rhs=d0e[:], start=True, stop=True
                    )
                    # r = 1/(1 + n)
                    n1 = wpool.tile([P, 1], f32, tag="n1")
                    nc.vector.tensor_scalar(
                        out=n1[:], in0=S[:, P : P + 1], scalar1=1.0, scalar2=None,
                        op0=mybir.AluOpType.add,
                    )
                    rv = wpool.tile([P, 1], f32, tag="rv")
                    nc.vector.reciprocal(out=rv[:], in_=n1[:])
                    # scatter value rows: -ALPHA * S * r
                    nc.vector.scalar_tensor_tensor(
                        out=sct[:, t, :],
                        in0=S[:, :P],
                        scalar=-ALPHA,
                        in1=rv[:].to_broadcast([P, P]),
                        op0=mybir.AluOpType.mult,
                        op1=mybir.AluOpType.mult,
                    )
                nc.gpsimd.dma_scatter_add(
                    out_ap=nctr[:],
                    in_ap=sct[:],
                    idxs_ap=six[:, kc * TPC * 8 : (kc + 1) * TPC * 8],
                    num_idxs=TPC * P,
                    num_idxs_reg=TPC * P,
                    elem_size=P,
                    queue_num=3,
                )
            nc.scalar.dma_start(out=lout[:], in_=losbuf[:])
    nc.finalize()
    return nc


# ----------------------------------------------------------------------------
# host sharding / packing
# ----------------------------------------------------------------------------

def host_pack(labels: np.ndarray, ncores: int, csh: int):
    """Sort by label, range-shard, pack into straddle-free 128-slot tiles.

    Returns (cores metadata list, s_pad). Pure index manipulation.
    """
    labels = np.asarray(labels).reshape(-1).astype(np.int64)
    order = np.argsort(labels, kind="stable")
    slab = labels[order]
    bounds = np.searchsorted(slab, np.arange(ncores + 1) * csh)
    packed = []
    used_max = 0
    for c in range(ncores):
        lo, hi = bounds[c], bounds[c + 1]
        samp = order[lo:hi]            # original sample idx, sorted by label
        lab = slab[lo:hi] - c * csh    # local labels, ascending
        n = lab.shape[0]
        starts = np.flatnonzero(np.r_[True, lab[1:] != lab[:-1]])
        lens = np.diff(np.r_[starts, n])
        assert lens.max(initial=0) <= P, "class run exceeds one tile"
        slot = np.empty(n, np.int64)
        cur = 0
        for s, L in zip(starts.tolist(), lens.tolist()):
            room = P - (cur % P)
            if L > room:
                cur += room
            slot[s : s + L] = np.arange(cur, cur + L)
            cur += L
        packed.append((samp, lab, starts, slot, cur))
        used_max = max(used_max, cur)

    s_pad = -(-used_max // 2048) * 2048
    cores = []
    for c in range(ncores):
        samp, lab, starts, slot, cur = packed[c]
        samp_at = np.full(s_pad, -1, np.int64)
        samp_at[slot] = samp
        real = samp_at >= 0

        gidx = np.zeros(s_pad, np.int16)
        gidx[slot] = lab.astype(np.int16)

        # first-slot-offset within tile, per slot; pads are singletons
        sl = np.arange(s_pad)
        fo = (sl % P).astype(np.int64)
        fo_real = np.empty(len(slot), np.int64)
        fo_real[:] = slot[starts].repeat(np.diff(np.r_[starts, len(slot)]))
        fo[slot] = fo_real % P

        first = np.zeros(s_pad, bool)
        first[slot[starts]] = True

        cores.append(
            dict(samp_at=samp_at, real=real, gidx=gidx,
                 fo=fo.astype(np.float32), first=first,
                 lab_first=lab[starts].astype(np.int16),
                 slot_first=slot[starts])
        )
    return cores, s_pad


def _wrap_idx(a: np.ndarray) -> np.ndarray:
    """[S] int16 -> [128, S/16] wrapped layout replicated to 8 groups."""
    w = a.reshape(-1, 16).T  # [16, S/16]
    return np.tile(w, (8, 1)).copy()


def make_in_maps(features, centers, cores, cfg: Cfg):
    features = np.asarray(features, dtype=np.float32)
    centers = np.asarray(centers, dtype=np.float32)
    T = cfg.n_tiles
    in_maps = []
    iota = np.tile(np.arange(P, dtype=np.float32), (P, 1))
    for c, m in enumerate(cores):
        fs = np.zeros((cfg.s_pad, P), np.float32)
        fs[m["real"]] = features[m["samp_at"][m["real"]]]
        feat_sw = np.ascontiguousarray(
            fs.reshape(T, P, P).transpose(1, 0, 2).reshape(P, T * P)
        )
        ctab = np.zeros((cfg.tbl, P), np.float32)
        ctab[: cfg.csh] = centers[c * cfg.csh : (c + 1) * cfg.csh]

        sl = np.arange(cfg.s_pad)
        sct = (cfg.csh + (sl % cfg.dump)).astype(np.int16)
        sct[m["slot_first"]] = m["lab_first"]

        in_maps.append(
            {
                "feat": feat_sw,
                "ctr": ctab,
                "gidx": _wrap_idx(m["gidx"]),
                "sidx": _wrap_idx(sct),
                "foT": np.ascontiguousarray(m["fo"].reshape(T, P).T),
                "iota": iota,
            }
        )
    return in_maps


def unshard(results, cores, cfg: Cfg):
    result = np.empty((cfg.B, 1), np.float32)
    new_centers = np.empty((cfg.C, P), np.float32)
    for c, (res, m) in enumerate(zip(results, cores)):
        new_centers[c * cfg.csh : (c + 1) * cfg.csh] = res["nctr"][: cfg.csh]
        loss_sorted = res["lout"].T.reshape(cfg.s_pad)  # slot i = [i%128, i//128]
        real = m["real"]
        result[m["samp_at"][real], 0] = loss_sorted[real]
    return result, new_centers


# ----------------------------------------------------------------------------
# entry point
# ----------------------------------------------------------------------------

_NC_CACHE = {}


def _get_nc(cfg: Cfg):
    key = (cfg.C, cfg.B, cfg.s_pad)
    if key not in _NC_CACHE:
        _NC_CACHE[key] = build_program(cfg)
    return _NC_CACHE[key]


def run(features, labels, centers, num_classes=NUM_CLASSES, dump_rows=1536,
        **spmd_kwargs):
    cores, s_pad = host_pack(labels, NCORES, num_classes // NCORES)
    cfg = Cfg(num_classes, len(np.asarray(labels).reshape(-1)), NCORES, s_pad,
              dump_rows=dump_rows)
    in_maps = make_in_maps(features, centers, cores, cfg)
    nc = _get_nc(cfg)
    br = run_bass_kernel_spmd(nc, in_maps, list(range(cfg.ncores)), **spmd_kwargs)
    result, new_centers = unshard(br.results, cores, cfg)
    return result, new_centers, br


def kernel(features, labels, centers):
    result, new_centers, _ = run(features, labels, centers)
    return result, new_centers
